# revision 14
# baseline (speedup 1.0000x reference)
"""CrossAttentionBlock kernel for 8 Trainium2 NeuronCores.

Reference computation (per batch b):
    q = x @ Wq;  k,v = y @ Wkv;  per head: softmax(q k^T / sqrt(dk)) v;
    out = concat_heads @ Wproj + bproj

Sharding: 8 cores = 2 batches x 4 head-groups (4 heads each). Each core
computes the partial output contribution of its 4 heads for its batch;
the host sums the 4 partials per batch and adds the bias.

Per-core layout (host prepares):
    xT  [1024, 2048]  x[b].T            (contraction dim on partitions)
    yT  [768, 2048]   y[b].T
    wq  [1024, 256]   Wq columns of this head group
    wk  [768, 256]    K-half of Wkv for this head group
    wv  [768, 256]    V-half of Wkv for this head group
    wp  [256, 1024]   Wproj rows of this head group
Output:
    outT [1024, 2048] partial (x @ .. @ Wproj).T for this head group

All matmuls run in float32r (full PE rate). PSUM accumulation is fp32.
"""

import numpy as np

import concourse.bass as bass
import concourse.tile as tile
from concourse import bacc, mybir
from concourse.bass_utils import run_bass_kernel_spmd

B, LQ, LKV = 2, 2048, 2048
C, CTX, H, DK = 1024, 768, 16, 64
SCALE = DK ** (-0.5)

F32 = mybir.dt.float32
F32R = mybir.dt.float32r


def _bcast_rows(ap: bass.AP, nrows: int) -> bass.AP:
    """AP that reads a single-partition row `nrows` times (partition step 0)."""
    assert ap.ap[0][1] == 1, ap.ap
    return bass.AP(tensor=ap.tensor, offset=ap.offset, ap=[[0, nrows]] + ap.ap[1:])


def build_kernel(lq=LQ, lkv=LKV, c=C, ctx=CTX, hd=256, debug_taps=False):
    """One core's program: 4 heads (2 pairs) of cross-attention + partial proj."""
    nc = bacc.Bacc("TRN2", target_bir_lowering=False, debug=False)

    xT = nc.dram_tensor("xT", [c, lq], F32, kind="ExternalInput").ap()
    yT = nc.dram_tensor("yT", [ctx, lkv], F32, kind="ExternalInput").ap()
    wq = nc.dram_tensor("wq", [c, hd], F32, kind="ExternalInput").ap()
    wk = nc.dram_tensor("wk", [ctx, hd], F32, kind="ExternalInput").ap()
    wv = nc.dram_tensor("wv", [ctx, hd], F32, kind="ExternalInput").ap()
    wp = nc.dram_tensor("wp", [hd, c], F32, kind="ExternalInput").ap()
    outT = nc.dram_tensor("outT", [c, lq], F32, kind="ExternalOutput").ap()
    # DRAM bounce buffer for the per-row 1/rowsum broadcast (SBUF APs cannot
    # have partition step 0; DRAM APs can)
    rsd = nc.dram_tensor("rsd", [hd // 128, lq // 512, 2, 512], F32,
                         kind="Internal").ap()
    taps = {}
    if debug_taps:
        taps["dbg_qt"] = nc.dram_tensor(
            "dbg_qt", [128, hd // 128, lq], F32, kind="ExternalOutput").ap()
        taps["dbg_kt"] = nc.dram_tensor(
            "dbg_kt", [128, hd // 128, lkv], F32, kind="ExternalOutput").ap()
        taps["dbg_vaug"] = nc.dram_tensor(
            "dbg_vaug", [128, lkv // 128, 4, 65], F32, kind="ExternalOutput").ap()
        taps["dbg_rs"] = nc.dram_tensor(
            "dbg_rs", [hd // 128, lq // 512, 2, 512], F32, kind="ExternalOutput").ap()
        taps["dbg_otn"] = nc.dram_tensor(
            "dbg_otn", [128, hd // 128, lq], F32, kind="ExternalOutput").ap()

    ncc = c // 128          # contraction chunks for Q proj (8)
    nctx = ctx // 128       # contraction chunks for K/V proj (6)
    nit = lq // 512         # i tiles (4)
    njt = lkv // 128        # j chunks (16)
    npair = hd // 128       # head pairs (2)
    nct = c // 128          # out column tiles (8)

    with tile.TileContext(nc) as tc:
        with (
            tc.tile_pool(name="big", bufs=1) as big,
            tc.tile_pool(name="wts", bufs=1) as wts,
            tc.tile_pool(name="acts", bufs=1) as acts,
            tc.tile_pool(name="pt", bufs=2) as ptp,
            tc.tile_pool(name="nrm", bufs=4) as nrm,
            tc.tile_pool(name="stg", bufs=2) as stgp,
            tc.tile_pool(name="osb", bufs=3) as osb,
            tc.tile_pool(name="st", bufs=1, space="PSUM") as stp,
            tc.tile_pool(name="ot", bufs=2, space="PSUM") as otp,
        ):
            # ---- persistent activations/weights in SBUF
            qt = acts.tile([128, npair, lq], F32R, tag="qt")      # Q^T pair-stacked
            kt = acts.tile([128, npair, lkv], F32R, tag="kt")     # K^T pair-stacked
            vaug = acts.tile([128, njt, 4, 65], F32R, tag="vaug")  # [V_h | ones] per j-chunk
            otn = acts.tile([128, npair, lq], F32R, tag="otn")    # normalized O^T

            # ---- phase A: Q projection (qt[hd, lq] = wq.T @ x.T)
            x_sb = big.tile([128, ncc, lq], F32R, tag="xy")
            nc.sync.dma_start(
                out=x_sb, in_=xT.rearrange("(cc p) l -> p cc l", p=128).bitcast(F32R))
            wq_sb = wts.tile([128, ncc, hd], F32R, tag="wq")
            nc.sync.dma_start(
                out=wq_sb, in_=wq.rearrange("(cc p) h -> p cc h", p=128).bitcast(F32R))

            for pair in range(npair):
                for it in range(nit):
                    ps = otp.tile([128, 512], F32, tag="ot")
                    for cc in range(ncc):
                        nc.tensor.matmul(
                            ps[:],
                            wq_sb[:, cc, pair * 128:(pair + 1) * 128],
                            x_sb[:, cc, it * 512:(it + 1) * 512],
                            start=(cc == 0), stop=(cc == ncc - 1))
                    nc.vector.tensor_copy(qt[:, pair, it * 512:(it + 1) * 512], ps[:])

            # ---- phase B: K projection and V projection
            y_sb = big.tile([128, nctx, lkv], F32R, tag="xy")
            nc.sync.dma_start(
                out=y_sb, in_=yT.rearrange("(cc p) l -> p cc l", p=128).bitcast(F32R))
            wk_sb = wts.tile([128, nctx, hd], F32R, tag="wk")
            nc.sync.dma_start(
                out=wk_sb, in_=wk.rearrange("(cc p) h -> p cc h", p=128).bitcast(F32R))
            wv_sb = wts.tile([128, nctx, hd], F32R, tag="wv")
            nc.sync.dma_start(
                out=wv_sb, in_=wv.rearrange("(cc p) h -> p cc h", p=128).bitcast(F32R))

            for pair in range(npair):
                for it in range(nit):
                    ps = otp.tile([128, 512], F32, tag="ot")
                    for cc in range(nctx):
                        nc.tensor.matmul(
                            ps[:],
                            wk_sb[:, cc, pair * 128:(pair + 1) * 128],
                            y_sb[:, cc, it * 512:(it + 1) * 512],
                            start=(cc == 0), stop=(cc == nctx - 1))
                    nc.vector.tensor_copy(kt[:, pair, it * 512:(it + 1) * 512], ps[:])

            ones_sb = wts.tile([128, njt, 4], F32, tag="ones")
            nc.vector.memset(ones_sb[:], 1.0)
            nc.vector.tensor_copy(
                vaug[:, :, :, 64:65],
                ones_sb[:].rearrange("p j (h o) -> p j h o", o=1))
            for jt in range(njt):
                ps = otp.tile([128, 256], F32, tag="ot")
                for cc in range(nctx):
                    nc.tensor.matmul(
                        ps[:],
                        y_sb[:, cc, jt * 128:(jt + 1) * 128],
                        wv_sb[:, cc, :],
                        start=(cc == 0), stop=(cc == nctx - 1))
                nc.vector.tensor_copy(
                    vaug[:, jt, :, 0:64],
                    ps[:].rearrange("p (h d) -> p h d", d=64))

            # ---- phase C: attention, per pair / i-tile; flash-style over j
            gmax = min(3, njt)
            groups = [(g0, min(gmax, njt - g0)) for g0 in range(0, njt, gmax)]
            for pair in range(npair):
                ha, hb = 2 * pair, 2 * pair + 1
                for it in range(nit):
                    ot_a = otp.tile([65, 512], F32, tag="ot")
                    ot_b = otp.tile([65, 512], F32, tag="ot")
                    for (g0, glen) in groups:
                        st = stp.tile([128, 2, glen, 512], F32, tag="st")
                        for k in range(glen):
                            jt = g0 + k
                            nc.tensor.matmul(
                                st[:, 0, k, :],
                                kt[0:64, pair, jt * 128:(jt + 1) * 128],
                                qt[0:64, pair, it * 512:(it + 1) * 512],
                                start=True, stop=True)
                            nc.tensor.matmul(
                                st[:, 1, k, :],
                                kt[64:128, pair, jt * 128:(jt + 1) * 128],
                                qt[64:128, pair, it * 512:(it + 1) * 512],
                                start=True, stop=True)
                        pt = ptp.tile([128, 2, gmax, 512], F32R, tag="pt")
                        nc.scalar.activation(
                            pt[:, :, 0:glen, :], st[:],
                            mybir.ActivationFunctionType.Exp, scale=SCALE)
                        for k in range(glen):
                            jt = g0 + k
                            nc.tensor.matmul(
                                ot_a[:], vaug[:, jt, ha, :], pt[:, 0, k, :],
                                start=(jt == 0), stop=(jt == njt - 1))
                            nc.tensor.matmul(
                                ot_b[:], vaug[:, jt, hb, :], pt[:, 1, k, :],
                                start=(jt == 0), stop=(jt == njt - 1))
                    # normalize: O^T[h] / rowsum (row 64 of each ot tile)
                    for h, ot in ((0, ot_a), (1, ot_b)):
                        rs = nrm.tile([65, 512], F32, tag="rs")
                        nc.vector.tensor_copy(rs[64:65, :], ot[64:65, :])
                        if debug_taps:
                            nc.sync.dma_start(out=taps["dbg_rs"][pair, it, h, :],
                                              in_=rs[64:65, :])
                        nc.vector.reciprocal(
                            out=rs[64:65, :], in_=rs[64:65, :])
                        nc.sync.dma_start(out=rsd[pair, it, h, :],
                                          in_=rs[64:65, :])
                        rc = nrm.tile([64, 512], F32, tag="rc")
                        nc.sync.dma_start(
                            out=rc, in_=_bcast_rows(rsd[pair, it, h:h + 1, :], 64))
                        if h == 0:
                            nc.vector.tensor_mul(
                                otn[0:64, pair, it * 512:(it + 1) * 512],
                                ot[0:64, :], rc[:])
                        else:
                            stg = stgp.tile([64, 512], F32R, tag="stg")
                            nc.vector.tensor_mul(stg[:], ot[0:64, :], rc[:])
                            nc.sync.dma_start(
                                out=otn[64:128, pair, it * 512:(it + 1) * 512],
                                in_=stg[:])

            if debug_taps:
                nc.sync.dma_start(out=taps["dbg_qt"], in_=qt[:].bitcast(F32))
                nc.sync.dma_start(out=taps["dbg_kt"], in_=kt[:].bitcast(F32))
                nc.sync.dma_start(out=taps["dbg_vaug"], in_=vaug[:].bitcast(F32))
                nc.sync.dma_start(out=taps["dbg_otn"], in_=otn[:].bitcast(F32))

            # ---- phase D: output projection outT[ct, it] += wp.T @ otn
            wp_sb = wts.tile([128, npair, c], F32R, tag="wp")
            nc.sync.dma_start(
                out=wp_sb, in_=wp.rearrange("(r p) o -> p r o", p=128).bitcast(F32R))
            for ct in range(nct):
                for it in range(nit):
                    ps = otp.tile([128, 512], F32, tag="ot")
                    for pair in range(npair):
                        nc.tensor.matmul(
                            ps[:],
                            wp_sb[:, pair, ct * 128:(ct + 1) * 128],
                            otn[:, pair, it * 512:(it + 1) * 512],
                            start=(pair == 0), stop=(pair == npair - 1))
                    o_sb = osb.tile([128, 512], F32, tag="osb")
                    nc.vector.tensor_copy(o_sb[:], ps[:])
                    nc.sync.dma_start(
                        out=outT[ct * 128:(ct + 1) * 128, it * 512:(it + 1) * 512],
                        in_=o_sb[:])

    nc.compile()
    return nc


_NC_CACHE = {}


def _get_nc():
    if "nc" not in _NC_CACHE:
        _NC_CACHE["nc"] = build_kernel()
    return _NC_CACHE["nc"]


def make_in_maps(x, y, Wq, Wkv, Wproj):
    """Host-side sharding: core = b * 4 + hg (hg = 4-head group)."""
    x = np.asarray(x, dtype=np.float32)
    y = np.asarray(y, dtype=np.float32)
    Wq = np.asarray(Wq, dtype=np.float32)
    Wkv = np.asarray(Wkv, dtype=np.float32).reshape(CTX, 2, H, DK)
    Wproj = np.asarray(Wproj, dtype=np.float32)

    in_maps = []
    for core in range(8):
        b, hg = core // 4, core % 4
        hs = slice(4 * hg, 4 * hg + 4)
        in_maps.append({
            "xT": np.ascontiguousarray(x[b].T),
            "yT": np.ascontiguousarray(y[b].T),
            "wq": np.ascontiguousarray(Wq[:, 4 * hg * DK:(4 * hg + 4) * DK]),
            "wk": np.ascontiguousarray(Wkv[:, 0, hs, :].reshape(CTX, 4 * DK)),
            "wv": np.ascontiguousarray(Wkv[:, 1, hs, :].reshape(CTX, 4 * DK)),
            "wp": np.ascontiguousarray(Wproj[4 * hg * DK:(4 * hg + 4) * DK, :]),
        })
    return in_maps


def kernel(x, y, Wq, Wkv, Wproj, bproj):
    nc = _get_nc()
    in_maps = make_in_maps(x, y, Wq, Wkv, Wproj)
    res = run_bass_kernel_spmd(nc, in_maps, core_ids=list(range(8)))
    bproj = np.asarray(bproj, dtype=np.float32)
    out = np.empty((B, LQ, C), dtype=np.float32)
    for b in range(B):
        acc = res.results[4 * b]["outT"].astype(np.float32).copy()
        for hg in range(1, 4):
            acc += res.results[4 * b + hg]["outT"]
        out[b] = acc.T + bproj
    return out
